# revision 11
# baseline (speedup 1.0000x reference)
"""Cross-attention fusion kernel for Trainium2 (8 NeuronCores).

Reference computation (per sample b):
    q = Wq @ xs + bq            xs = x_s2[b] as [256, 4096]
    k = Wk @ xd + bk            xd = x_dem[b] as [64, 4096]
    v = Wv @ xd + bv
    attn = softmax_j(k^T q * c)             c = 256 ** -0.5
    out = v @ attn + x_s2[b]                out[ch, j] = sum_i v[ch, i] attn[i, j]

Device-side restructure (mathematically identical):
  - kq = (Wq * c)^T @ k, so logits = kq^T @ xs and q never materializes.
  - bq adds a per-i constant to logits, which cancels in softmax_j -> dropped.
  - bk / bv folded in by augmenting xd with a ones row (contraction K=65).
  - softmax denominators folded into v columns (scale v[:, i] by 1/sum_j e[i, j])
    instead of scaling the whole e matrix.
  - exp logits are left unshifted (no running-max): logits are O(1) here and
    fp32 exp has ~1e38 of headroom.

Sharding: 8 cores = 4 samples x 2 halves of the key-pixel axis i. Each core
emits a partial out [256, 4096]; the host sums the two halves and adds the
residual. No collectives.
"""

import numpy as np
import ml_dtypes

import concourse.bass as bass
import concourse.mybir as mybir
import concourse.tile as tile
from concourse import bacc
from concourse.bass_utils import run_bass_kernel_spmd

P = 128
CH = 256          # out_ch == s2_ch
DEM = 64          # dem_ch
N = 4096          # pixels per sample (j axis)
NI = 2048         # key pixels per core (i axis, half of N)
KO = CH // P      # 2 partition chunks of the 256-channel axis
NIB = NI // P     # 16 i-blocks per core
NCORES = 8

F32 = mybir.dt.float32
BF16 = mybir.dt.bfloat16
NP_BF16 = ml_dtypes.bfloat16


def build_bass():
    nc = bacc.Bacc(None, target_bir_lowering=False)

    xs_d = nc.dram_tensor("xs", [CH, N], BF16, kind="ExternalInput")
    xda_d = nc.dram_tensor("xda", [DEM + 1, NI], BF16, kind="ExternalInput")
    wq_d = nc.dram_tensor("wq", [CH, CH], BF16, kind="ExternalInput")
    wk_d = nc.dram_tensor("wk", [DEM + 1, CH], BF16, kind="ExternalInput")
    wv_d = nc.dram_tensor("wv", [DEM + 1, CH], BF16, kind="ExternalInput")
    out_d = nc.dram_tensor("out", [CH, N], F32, kind="ExternalOutput")

    xs_v = xs_d.ap().rearrange("(ko p) j -> p ko j", p=P)
    wq_v = wq_d.ap().rearrange("(ko p) m -> p ko m", p=P)
    out_v = out_d.ap().rearrange("(m p) j -> p m j", p=P)

    with tile.TileContext(nc) as tc:
        with (
            tc.tile_pool(name="consts", bufs=1) as consts,
            tc.tile_pool(name="bigs", bufs=1) as bigs,
            tc.tile_pool(name="small", bufs=1) as small,
            tc.tile_pool(name="stage", bufs=4) as stage,
            tc.tile_pool(name="mm_psum", bufs=4, space="PSUM") as mm_psum,
            tc.tile_pool(name="out_psum", bufs=4, space="PSUM") as out_psum,
        ):
            wk_sb = consts.tile([DEM + 1, CH], BF16)
            nc.sync.dma_start(out=wk_sb, in_=wk_d.ap())
            wv_sb = consts.tile([DEM + 1, CH], BF16)
            nc.sync.dma_start(out=wv_sb, in_=wv_d.ap())
            wq_sb = consts.tile([P, KO, CH], BF16)
            nc.sync.dma_start(out=wq_sb, in_=wq_v)
            xda_sb = consts.tile([DEM + 1, NI], BF16)
            nc.sync.dma_start(out=xda_sb, in_=xda_d.ap())

            xs_sb = bigs.tile([P, KO, N], BF16)
            for ko in range(KO):
                nc.sync.dma_start(out=xs_sb[:, ko, :], in_=xs_v[:, ko, :])

            k_sb = bigs.tile([P, KO, NI], BF16)    # k[o, i], o on partitions
            kq_sb = bigs.tile([P, KO, NI], BF16)   # kq[ci, i], ci on partitions
            vt_sb = bigs.tile([P, NIB, CH], BF16)  # v^T[i, ch], i on partitions
            e_sb = bigs.tile([P, NIB, N], BF16)    # exp(logits)[i, j]

            sums_sb = small.tile([P, NIB, N // 512], F32)
            r_sb = small.tile([P, NIB], F32)

            # ---- Phase A: k = [Wk^T; bk]^T @ [xd; 1]  -> k_sb [o, i] ----
            for m in range(KO):
                for i4 in range(NI // 512):
                    ps = mm_psum.tile([P, 512], F32, tag="ps")
                    psl = ps[:, :512]
                    nc.tensor.matmul(
                        psl,
                        lhsT=wk_sb[:, m * P:(m + 1) * P],
                        rhs=xda_sb[:, i4 * 512:(i4 + 1) * 512],
                        start=True, stop=True,
                    )
                    nc.vector.tensor_copy(
                        out=k_sb[:, m, i4 * 512:(i4 + 1) * 512], in_=psl
                    )

            # ---- Phase B: v^T = [xd; 1]^T @ [Wv^T; bv] -> vt_sb [i, ch] ----
            for ib in range(NIB):
                ps = mm_psum.tile([P, 512], F32, tag="ps")
                psl = ps[:, :CH]
                nc.tensor.matmul(
                    psl,
                    lhsT=xda_sb[:, ib * P:(ib + 1) * P],
                    rhs=wv_sb,
                    start=True, stop=True,
                )
                nc.vector.tensor_copy(out=vt_sb[:, ib, :], in_=psl)

            # ---- Phase C: kq[ci, i] = sum_o (Wq*c)[o, ci] k[o, i] ----
            for m in range(KO):
                for i4 in range(NI // 512):
                    ps = mm_psum.tile([P, 512], F32, tag="ps")
                    s0 = i4 * 512
                    for ko in range(KO):
                        nc.tensor.matmul(
                            ps,
                            lhsT=wq_sb[:, ko, m * P:(m + 1) * P],
                            rhs=k_sb[:, ko, s0:s0 + 512],
                            start=(ko == 0), stop=(ko == 1),
                        )
                    nc.vector.tensor_copy(
                        out=kq_sb[:, m, s0:s0 + 512], in_=ps
                    )

            # ---- Phase D: logits -> exp -> row sums -> scale v^T rows ----
            for ib in range(NIB):
                for jn in range(N // 512):
                    pp = mm_psum.tile([P, 512], F32, tag="ps")
                    j0 = jn * 512
                    for ko in range(KO):
                        nc.tensor.matmul(
                            pp,
                            lhsT=kq_sb[:, ko, ib * P:(ib + 1) * P],
                            rhs=xs_sb[:, ko, j0:j0 + 512],
                            start=(ko == 0), stop=(ko == 1),
                        )
                    nc.scalar.activation(
                        out=e_sb[:, ib, j0:j0 + 512],
                        in_=pp,
                        func=mybir.ActivationFunctionType.Exp,
                        accum_out=sums_sb[:, ib, jn:jn + 1],
                    )
                nc.vector.reduce_sum(
                    out=r_sb[:, ib:ib + 1],
                    in_=sums_sb[:, ib, :],
                    axis=mybir.AxisListType.X,
                )
                nc.vector.reciprocal(out=r_sb[:, ib:ib + 1], in_=r_sb[:, ib:ib + 1])
                nc.vector.tensor_scalar_mul(
                    out=vt_sb[:, ib, :],
                    in0=vt_sb[:, ib, :],
                    scalar1=r_sb[:, ib:ib + 1],
                )

            # ---- Phase E: out[ch, j] = sum_i vts[i, ch] e[i, j] ----
            for jq in range(4):
                pq = [out_psum.tile([P, 512], F32, tag="po", name=f"po_{jq}_{t}")
                      for t in range(4)]
                for ib in range(NIB):
                    for m in range(KO):
                        for jj in range(2):
                            jn = jq * 2 + jj
                            nc.tensor.matmul(
                                pq[m * 2 + jj],
                                lhsT=vt_sb[:, ib, m * P:(m + 1) * P],
                                rhs=e_sb[:, ib, jn * 512:(jn + 1) * 512],
                                start=(ib == 0), stop=(ib == NIB - 1),
                            )
                for m in range(KO):
                    for jj in range(2):
                        jn = jq * 2 + jj
                        st = stage.tile([P, 512], F32, tag="st")
                        nc.vector.tensor_copy(out=st, in_=pq[m * 2 + jj])
                        nc.sync.dma_start(
                            out=out_v[:, m, jn * 512:(jn + 1) * 512], in_=st
                        )
    nc.finalize()
    return nc


_NC_CACHE = None


def _get_nc():
    global _NC_CACHE
    if _NC_CACHE is None:
        _NC_CACHE = build_bass()
    return _NC_CACHE


def make_in_maps(x_s2, x_dem, Wq, bq, Wk, bk, Wv, bv):
    scale = np.float32(CH ** -0.5)
    wq = np.ascontiguousarray(Wq * scale).astype(NP_BF16)                # [o, ci]
    wk = np.concatenate([Wk.T, bk[None, :]], axis=0).astype(NP_BF16)     # [65, 256]
    wv = np.concatenate([Wv.T, bv[None, :]], axis=0).astype(NP_BF16)
    ones = np.ones((1, NI), np.float32)
    in_maps = []
    for c in range(NCORES):
        s, h = divmod(c, 2)
        xs = np.ascontiguousarray(x_s2[s].reshape(CH, N)).astype(NP_BF16)
        xd = x_dem[s].reshape(DEM, N)[:, h * NI:(h + 1) * NI]
        xda = np.concatenate([xd, ones], axis=0).astype(NP_BF16)
        in_maps.append({"xs": xs, "xda": np.ascontiguousarray(xda),
                        "wq": wq, "wk": wk, "wv": wv})
    return in_maps


def run(inputs, trace=False, trace_cores=None):
    """Run the device kernel; returns (output, BassKernelResults)."""
    x_s2 = np.asarray(inputs["x_s2"], np.float32)
    x_dem = np.asarray(inputs["x_dem"], np.float32)
    args = {k: np.asarray(inputs[k], np.float32)
            for k in ("Wq", "bq", "Wk", "bk", "Wv", "bv")}
    in_maps = make_in_maps(x_s2, x_dem, args["Wq"], args["bq"],
                           args["Wk"], args["bk"], args["Wv"], args["bv"])
    nc = _get_nc()
    res = run_bass_kernel_spmd(nc, in_maps, core_ids=list(range(NCORES)),
                               trace=trace, trace_cores=trace_cores)
    B = x_s2.shape[0]
    out = np.empty_like(x_s2)
    for s in range(B):
        part = res.results[2 * s]["out"] + res.results[2 * s + 1]["out"]
        out[s] = part.reshape(CH, 64, 64) + x_s2[s]
    return out, res


def kernel(**inputs):
    out, _ = run(inputs, trace=False)
    return out


# revision 18
# speedup vs baseline: 1.1197x; 1.1197x over previous
"""Cross-attention fusion kernel for Trainium2 (8 NeuronCores).

Reference computation (per sample b):
    q = Wq @ xs + bq            xs = x_s2[b] as [256, 4096]
    k = Wk @ xd + bk            xd = x_dem[b] as [64, 4096]
    v = Wv @ xd + bv
    attn = softmax_j(k^T q * c)             c = 256 ** -0.5
    out = v @ attn + x_s2[b]                out[ch, j] = sum_i v[ch, i] attn[i, j]

Device-side restructure (mathematically identical):
  - kq = (Wq * c)^T @ k, so logits = kq^T @ xs and q never materializes.
  - bq adds a per-i constant to logits, which cancels in softmax_j -> dropped.
  - bk / bv folded in by augmenting xd with a ones row (contraction K=65).
  - softmax denominators folded into v columns (scale v[:, i] by 1/sum_j e[i, j])
    instead of scaling the whole e matrix.
  - exp logits are left unshifted (no running-max): logits are O(1) here and
    fp32 exp has ~1e38 of headroom.

Sharding: 8 cores = 4 samples x 2 halves of the key-pixel axis i. Each core
emits a partial out [256, 4096]; the host sums the two halves and adds the
residual. No collectives.
"""

import numpy as np
import ml_dtypes

import concourse.bass as bass
import concourse.mybir as mybir
import concourse.tile as tile
from concourse import bacc
from concourse.bass_utils import run_bass_kernel_spmd

P = 128
CH = 256          # out_ch == s2_ch
DEM = 64          # dem_ch
N = 4096          # pixels per sample (j axis)
NI = 2048         # key pixels per core (i axis, half of N)
KO = CH // P      # 2 partition chunks of the 256-channel axis
NIB = NI // P     # 16 i-blocks per core
NCORES = 8

F32 = mybir.dt.float32
BF16 = mybir.dt.bfloat16
NP_BF16 = ml_dtypes.bfloat16


def build_bass():
    nc = bacc.Bacc(None, target_bir_lowering=False)

    xs_d = nc.dram_tensor("xs", [CH, N], BF16, kind="ExternalInput")
    xda_d = nc.dram_tensor("xda", [DEM + 1, NI], BF16, kind="ExternalInput")
    wq_d = nc.dram_tensor("wq", [CH, CH], BF16, kind="ExternalInput")
    wk_d = nc.dram_tensor("wk", [DEM + 1, CH], BF16, kind="ExternalInput")
    wv_d = nc.dram_tensor("wv", [DEM + 1, CH], BF16, kind="ExternalInput")
    out_d = nc.dram_tensor("out", [CH, N], F32, kind="ExternalOutput")

    xs_v = xs_d.ap().rearrange("(ko p) j -> p ko j", p=P)
    wq_v = wq_d.ap().rearrange("(ko p) m -> p ko m", p=P)
    out_v = out_d.ap().rearrange("(m p) j -> p m j", p=P)

    with tile.TileContext(nc) as tc:
        with (
            tc.tile_pool(name="consts", bufs=1) as consts,
            tc.tile_pool(name="bigs", bufs=1) as bigs,
            tc.tile_pool(name="small", bufs=1) as small,
            tc.tile_pool(name="stage", bufs=4) as stage,
            tc.tile_pool(name="mm_psum", bufs=2, space="PSUM") as mm_psum,
            tc.tile_pool(name="out_psum", bufs=4, space="PSUM") as out_psum,
        ):
            # Phase A needs only wk + xda: issue those DMAs first.
            wk_sb = consts.tile([DEM + 1, CH], BF16)
            nc.sync.dma_start(out=wk_sb, in_=wk_d.ap())
            xda_sb = consts.tile([DEM + 1, NI], BF16)
            nc.sync.dma_start(out=xda_sb, in_=xda_d.ap())
            wv_sb = consts.tile([DEM + 1, CH], BF16)
            nc.sync.dma_start(out=wv_sb, in_=wv_d.ap())
            wq_sb = consts.tile([P, KO, CH], BF16)
            nc.sync.dma_start(out=wq_sb, in_=wq_v)

            xs_sb = bigs.tile([P, KO, N], BF16)
            for ko in range(KO):
                nc.sync.dma_start(out=xs_sb[:, ko, :], in_=xs_v[:, ko, :])

            k_sb = bigs.tile([P, KO, NI], BF16)    # k[o, i], o on partitions
            kq_sb = bigs.tile([P, KO, NI], BF16)   # kq[ci, i], ci on partitions
            vt_sb = bigs.tile([P, NIB, CH], BF16)  # v^T[i, ch], i on partitions
            e_sb = bigs.tile([P, NIB, N], BF16)    # exp(logits)[i, j]

            r_sb = small.tile([P, NIB], F32)

            # ---- Phase A: k = [Wk^T; bk]^T @ [xd; 1]  -> k_sb [o, i] ----
            for m in range(KO):
                for i4 in range(NI // 512):
                    ps = mm_psum.tile([P, 1024], F32, tag="ps")
                    psl = ps[:, :512]
                    nc.tensor.matmul(
                        psl,
                        lhsT=wk_sb[:, m * P:(m + 1) * P],
                        rhs=xda_sb[:, i4 * 512:(i4 + 1) * 512],
                        start=True, stop=True,
                    )
                    nc.vector.tensor_copy(
                        out=k_sb[:, m, i4 * 512:(i4 + 1) * 512], in_=psl
                    )

            # ---- Phase B: v^T = [xd; 1]^T @ [Wv^T; bv] -> vt_sb [i, ch] ----
            for ib in range(NIB):
                ps = mm_psum.tile([P, 1024], F32, tag="ps")
                psl = ps[:, :CH]
                nc.tensor.matmul(
                    psl,
                    lhsT=xda_sb[:, ib * P:(ib + 1) * P],
                    rhs=wv_sb,
                    start=True, stop=True,
                )
                nc.vector.tensor_copy(out=vt_sb[:, ib, :], in_=psl)

            # ---- Phase C: kq[ci, i] = sum_o (Wq*c)[o, ci] k[o, i] ----
            for m in range(KO):
                for ip in range(NI // 1024):
                    ps = mm_psum.tile([P, 1024], F32, tag="ps")
                    for ko in range(KO):
                        for jj in range(2):
                            s0 = ip * 1024 + jj * 512
                            nc.tensor.matmul(
                                ps[:, jj * 512:(jj + 1) * 512],
                                lhsT=wq_sb[:, ko, m * P:(m + 1) * P],
                                rhs=k_sb[:, ko, s0:s0 + 512],
                                start=(ko == 0), stop=(ko == 1),
                            )
                    nc.vector.tensor_copy(
                        out=kq_sb[:, m, ip * 1024:(ip + 1) * 1024], in_=ps
                    )

            # ---- Phase D: logits -> exp -> row sums -> scale v^T rows ----
            for ib in range(NIB):
                for jp in range(N // 1024):
                    pp = mm_psum.tile([P, 1024], F32, tag="ps")
                    for ko in range(KO):
                        for jj in range(2):
                            j0 = jp * 1024 + jj * 512
                            nc.tensor.matmul(
                                pp[:, jj * 512:(jj + 1) * 512],
                                lhsT=kq_sb[:, ko, ib * P:(ib + 1) * P],
                                rhs=xs_sb[:, ko, j0:j0 + 512],
                                start=(ko == 0), stop=(ko == 1),
                            )
                    nc.scalar.activation(
                        out=e_sb[:, ib, jp * 1024:(jp + 1) * 1024],
                        in_=pp,
                        func=mybir.ActivationFunctionType.Exp,
                    )
                nc.vector.reduce_sum(
                    out=r_sb[:, ib:ib + 1],
                    in_=e_sb[:, ib, :],
                    axis=mybir.AxisListType.X,
                )
                nc.vector.reciprocal(out=r_sb[:, ib:ib + 1], in_=r_sb[:, ib:ib + 1])
                nc.vector.tensor_scalar_mul(
                    out=vt_sb[:, ib, :],
                    in0=vt_sb[:, ib, :],
                    scalar1=r_sb[:, ib:ib + 1],
                )

            # ---- Phase E: out[ch, j] = sum_i vts[i, ch] e[i, j] ----
            for jq in range(4):
                pq = [out_psum.tile([P, 512], F32, tag="po", name=f"po_{jq}_{t}")
                      for t in range(4)]
                for ib in range(NIB):
                    for m in range(KO):
                        for jj in range(2):
                            jn = jq * 2 + jj
                            nc.tensor.matmul(
                                pq[m * 2 + jj],
                                lhsT=vt_sb[:, ib, m * P:(m + 1) * P],
                                rhs=e_sb[:, ib, jn * 512:(jn + 1) * 512],
                                start=(ib == 0), stop=(ib == NIB - 1),
                            )
                for m in range(KO):
                    for jj in range(2):
                        jn = jq * 2 + jj
                        st = stage.tile([P, 512], F32, tag="st")
                        nc.vector.tensor_copy(out=st, in_=pq[m * 2 + jj])
                        nc.sync.dma_start(
                            out=out_v[:, m, jn * 512:(jn + 1) * 512], in_=st
                        )
    nc.finalize()
    return nc


_NC_CACHE = None


def _get_nc():
    global _NC_CACHE
    if _NC_CACHE is None:
        _NC_CACHE = build_bass()
    return _NC_CACHE


def make_in_maps(x_s2, x_dem, Wq, bq, Wk, bk, Wv, bv):
    scale = np.float32(CH ** -0.5)
    wq = np.ascontiguousarray(Wq * scale).astype(NP_BF16)                # [o, ci]
    wk = np.concatenate([Wk.T, bk[None, :]], axis=0).astype(NP_BF16)     # [65, 256]
    wv = np.concatenate([Wv.T, bv[None, :]], axis=0).astype(NP_BF16)
    ones = np.ones((1, NI), np.float32)
    in_maps = []
    for c in range(NCORES):
        s, h = divmod(c, 2)
        xs = np.ascontiguousarray(x_s2[s].reshape(CH, N)).astype(NP_BF16)
        xd = x_dem[s].reshape(DEM, N)[:, h * NI:(h + 1) * NI]
        xda = np.concatenate([xd, ones], axis=0).astype(NP_BF16)
        in_maps.append({"xs": xs, "xda": np.ascontiguousarray(xda),
                        "wq": wq, "wk": wk, "wv": wv})
    return in_maps


def run(inputs, trace=False, trace_cores=None):
    """Run the device kernel; returns (output, BassKernelResults)."""
    x_s2 = np.asarray(inputs["x_s2"], np.float32)
    x_dem = np.asarray(inputs["x_dem"], np.float32)
    args = {k: np.asarray(inputs[k], np.float32)
            for k in ("Wq", "bq", "Wk", "bk", "Wv", "bv")}
    in_maps = make_in_maps(x_s2, x_dem, args["Wq"], args["bq"],
                           args["Wk"], args["bk"], args["Wv"], args["bv"])
    nc = _get_nc()
    res = run_bass_kernel_spmd(nc, in_maps, core_ids=list(range(NCORES)),
                               trace=trace, trace_cores=trace_cores)
    B = x_s2.shape[0]
    out = np.empty_like(x_s2)
    for s in range(B):
        part = res.results[2 * s]["out"] + res.results[2 * s + 1]["out"]
        out[s] = part.reshape(CH, 64, 64) + x_s2[s]
    return out, res


def kernel(**inputs):
    out, _ = run(inputs, trace=False)
    return out
